# revision 22
# baseline (speedup 1.0000x reference)
"""Balanced CE loss + accuracy on 8 Trainium2 NeuronCores (Bass/Tile).

Reference computation (N = 16777216 elements):
    loss = -sum(where(t==1, 1.6*log(p), 0.4*log(1-p))) / N
    acc  = mean(round(p) == t)

Strategy (data-parallel over N, no collectives).  Measured engine facts
on this HW (from perfetto traces of prior variants):
  - DVE 2-input ops (STT) always run 1x (~1.08 ns/col); 1-input
    tensor_scalar runs 2x (~0.52 ns/col); the accumulating
    tensor_scalar variant drops to 1x, so counts are cheaper as plain
    is_ge masks reduced on the idle TensorE via ones^T matmuls.
  - ACT runs ~0.92 ns/col + 185 ns per accumulator read.
  - GpSimd elementwise work and de-serialized engine overlap both
    inflate op latencies ~1.2-2.4x (SBUF port contention), so Pool
    stays idle and Ln runs IN PLACE after the masks (the dependency
    chain spreads SBUF pressure; a fully parallel variant measured
    slower).
  - The runtime preamble costs a fixed ~6.7 us, each dma_start ~0.6 us
    of serial SP dispatch; the io pool is deep enough that DMA issue
    never waits on compute.

Per sub-chunk (the identity ln(1)=0 masks without a select):
    a1 = (p-1)*t = y1-1  ->  sum ln(y1) = sum_{t==1} ln(p)   =: A1
    a0 = (t-1)*p = y0-1  ->  sum ln(y0) = sum_{t==0} ln(1-p) =: B0
  each one fused DVE scalar_tensor_tensor (int32 t converted on read).
  ACT computes Ln(a+1) in place with fused free-dim accumulation.
  Accuracy from exact f32 threshold counts, C1+C0-N:
    C1 = #(a1 >= -.5) = N0 + #(t1,p>=.5): DVE is_ge -> bf16 mask,
         partition-reduced by TensorE (ones^T @ mask) into PSUM.
    C0 = #(a0 >= -.5): ACT Sign(ln(y0)+ln2) accum on the six middle
         2048-col sub-chunks (75% of columns, balancing ACT ~46 us vs
         DVE ~46 us), DVE is_ge masks on the four 1024-col end
         sub-chunks so the ends stay ACT-light.
  Sub-chunks are 1024 cols at both ends to shrink pipeline fill and
  drain; the work pool is 3 deep so DVE never stalls on ACT finishing
  a sub-chunk.  Activation biases are tracked tiles (no const-AP
  barrier) and the Ln table is pre-warmed behind the first DMA.
"""

import sys

if "/opt/trn_rl_repo" not in sys.path:
    sys.path.insert(0, "/opt/trn_rl_repo")

import numpy as np

import concourse.bass as bass
import concourse.bacc as bacc
import concourse.tile as tile
from concourse import mybir
from concourse.bass_utils import run_bass_kernel_spmd

N_CORES = 8
N = 16777216
P = 128
SHARD = N // N_CORES          # 2097152 elements per core
COLS = SHARD // P             # 16384 columns per core
SUBS = [1024, 1024, 2048, 2048, 2048, 2048, 2048, 2048, 1024, 1024]
assert sum(SUBS) == COLS
NS = len(SUBS)
# C0 via ACT Sign for the middle 2048-col subs, DVE is_ge on the ends
SIGN_SUBS = frozenset({2, 3, 4, 5, 6, 7})
MMCOL = 512                   # matmul free-dim tile (one PSUM bank)
LN2 = 0.6931471805599453

AF = mybir.ActivationFunctionType
OP = mybir.AluOpType

# acc columns: [s] A1 sums, [NS+s] B0, [2NS+s] sign C0 (+/-1 sums,
# unused cols stay 0), [3NS] C1 fold, [3NS+1] DVE-C0 fold
ACC_COLS = 3 * NS + 2

_NC_CACHE = None


def build_bass():
    """Build the single-core Bass program (SPMD across 8 cores)."""
    global _NC_CACHE
    if _NC_CACHE is not None:
        return _NC_CACHE

    nc = bacc.Bacc("TRN2", target_bir_lowering=False, debug=False)

    p_in = nc.dram_tensor("p_in", [SHARD], mybir.dt.float32, kind="ExternalInput").ap()
    t_in = nc.dram_tensor("t_in", [SHARD], mybir.dt.int32, kind="ExternalInput").ap()
    acc_out = nc.dram_tensor("acc_out", [P, ACC_COLS], mybir.dt.float32, kind="ExternalOutput").ap()

    n_mm1 = COLS // MMCOL                                        # C1 matmuls
    dve_c0_cols = sum(SUBS[s] for s in range(NS) if s not in SIGN_SUBS)
    n_mm0 = dve_c0_cols // MMCOL                                 # C0 matmuls

    with tile.TileContext(nc) as tc:
        with (
            tc.tile_pool(name="io", bufs=8) as io_pool,
            tc.tile_pool(name="wk", bufs=3) as wk_pool,
            tc.tile_pool(name="jk", bufs=2) as jk_pool,
            tc.tile_pool(name="ps", bufs=1, space=bass.MemorySpace.PSUM) as psum_pool,
            tc.tile_pool(name="mi", bufs=1) as misc_pool,
        ):
            ones16 = misc_pool.tile([P, P], mybir.dt.bfloat16, tag="ones16")
            nc.gpsimd.memset(ones16[:], 1.0)
            co = misc_pool.tile([P, 1], mybir.dt.float32, tag="co")
            ln2c = misc_pool.tile([P, 1], mybir.dt.float32, tag="ln2c")
            nc.gpsimd.memset(co[:], 1.0)
            nc.gpsimd.memset(ln2c[:], LN2)
            warm = misc_pool.tile([P, 1], mybir.dt.float32, tag="warm")
            acc = misc_pool.tile([P, ACC_COLS], mybir.dt.float32, tag="acc")
            nc.gpsimd.memset(acc[:], 0.0)
            junk512 = misc_pool.tile([P, MMCOL], mybir.dt.float32, tag="junk512")
            ps1 = psum_pool.tile([P, MMCOL], mybir.dt.float32, tag="ps1")
            ps0 = psum_pool.tile([P, MMCOL], mybir.dt.float32, tag="ps0")

            # load the Ln/Sign table while the first DMA is in flight
            nc.scalar.activation(warm[:, 0:1], co[:, 0:1], AF.Ln, bias=co[:, 0:1])

            mm1 = mm0 = 0
            off = 0
            for s, C in enumerate(SUBS):
                p_t = io_pool.tile([P, C], mybir.dt.float32, tag="p")
                t_t = io_pool.tile([P, C], mybir.dt.int32, tag="t")
                nc.sync.dma_start(p_t[:], p_in[off : off + C * P].rearrange("(p f) -> p f", p=P))
                nc.sync.dma_start(t_t[:], t_in[off : off + C * P].rearrange("(p f) -> p f", p=P))
                off += C * P

                a1 = wk_pool.tile([P, C], mybir.dt.float32, tag="a1")
                a0 = wk_pool.tile([P, C], mybir.dt.float32, tag="a0")
                # a1 = (p-1)*t ; a0 = (t-1)*p
                nc.vector.scalar_tensor_tensor(a1[:], p_t[:], -1.0, t_t[:], OP.add, OP.mult)
                nc.vector.scalar_tensor_tensor(a0[:], t_t[:], -1.0, p_t[:], OP.add, OP.mult)

                # C1 mask (a1 >= -.5) at 2x into bf16, reduced on TensorE
                j1 = jk_pool.tile([P, C], mybir.dt.bfloat16, tag="j1")
                nc.vector.tensor_scalar(j1[:], a1[:], -0.5, None, OP.is_ge)
                for j in range(C // MMCOL):
                    nc.tensor.matmul(ps1[:], ones16[:], j1[:, j * MMCOL : (j + 1) * MMCOL],
                                     start=(mm1 == 0), stop=(mm1 == n_mm1 - 1))
                    mm1 += 1
                if s not in SIGN_SUBS:
                    jc = jk_pool.tile([P, C], mybir.dt.bfloat16, tag="jc")
                    nc.vector.tensor_scalar(jc[:], a0[:], -0.5, None, OP.is_ge)
                    for j in range(C // MMCOL):
                        nc.tensor.matmul(ps0[:], ones16[:], jc[:, j * MMCOL : (j + 1) * MMCOL],
                                         start=(mm0 == 0), stop=(mm0 == n_mm0 - 1))
                        mm0 += 1

                # in-place Ln with fused free-dim accumulation
                nc.scalar.activation(a1[:], a1[:], AF.Ln, bias=co[:, 0:1],
                                     accum_out=acc[:, s : s + 1])
                nc.scalar.activation(a0[:], a0[:], AF.Ln, bias=co[:, 0:1],
                                     accum_out=acc[:, NS + s : NS + s + 1])
                if s in SIGN_SUBS:
                    # post-Ln: sign(ln(y0)+ln2) = +/-1 for y0 >= .5 / < .5
                    js = jk_pool.tile([P, C], mybir.dt.bfloat16, tag="jc")
                    nc.scalar.activation(js[:], a0[:], AF.Sign, bias=ln2c[:, 0:1],
                                         accum_out=acc[:, 2 * NS + s : 2 * NS + s + 1])

            # fold the PSUM count matrices (128 identical rows) into columns
            nc.vector.tensor_scalar(junk512[:], ps1[:], 1.0 / P, None, OP.mult,
                                    OP.add, accum_out=acc[:, 3 * NS : 3 * NS + 1])
            nc.vector.tensor_scalar(junk512[:], ps0[:], 1.0 / P, None, OP.mult,
                                    OP.add, accum_out=acc[:, 3 * NS + 1 : 3 * NS + 2])

            nc.sync.dma_start(acc_out[:], acc[:])

    nc.finalize()
    _NC_CACHE = nc
    return nc


def make_in_maps(input, target):
    inp = np.ascontiguousarray(np.asarray(input, dtype=np.float32)).reshape(
        N_CORES, SHARD
    )
    tgt = np.ascontiguousarray(np.asarray(target, dtype=np.int32)).reshape(
        N_CORES, SHARD
    )
    return [{"p_in": inp[c], "t_in": tgt[c]} for c in range(N_CORES)]


def combine(results):
    """Host-side unshard: reduce the 8 cores' partial sums -> (loss, acc)."""
    A1 = B0 = S0 = C1 = C0m = 0.0
    sign_elems = sum(SUBS[s] for s in SIGN_SUBS) * P
    for r in results:
        a = np.asarray(r["acc_out"], dtype=np.float64)
        A1 += a[:, 0:NS].sum()
        B0 += a[:, NS : 2 * NS].sum()
        S0 += a[:, 2 * NS : 3 * NS].sum()
        C1 += a[:, 3 * NS].sum()
        C0m += a[:, 3 * NS + 1].sum()
    loss = -(1.6 * A1 + 0.4 * B0) / N
    C0 = (S0 + N_CORES * sign_elems) / 2.0 + C0m
    acc = (C1 + C0 - N) / N
    return np.float32(loss), np.float32(acc)


def run_on_hw(input, target, **spmd_kwargs):
    nc = build_bass()
    in_maps = make_in_maps(input, target)
    return run_bass_kernel_spmd(nc, in_maps, list(range(N_CORES)), **spmd_kwargs)


def kernel(input, target):
    br = run_on_hw(input, target)
    return combine(br.results)


# revision 23
# speedup vs baseline: 1.2082x; 1.2082x over previous
"""Balanced CE loss + accuracy on 8 Trainium2 NeuronCores (Bass/Tile).

Reference computation (N = 16777216 elements):
    loss = -sum(where(t==1, 1.6*log(p), 0.4*log(1-p))) / N
    acc  = mean(round(p) == t)

Strategy (data-parallel over N, no collectives).  Measured engine facts
on this HW (from perfetto traces of prior variants):
  - DVE 2-input ops (STT) always run 1x (~1.08 ns/col); 1-input
    tensor_scalar runs 2x (~0.52 ns/col); the accumulating
    tensor_scalar variant drops to 1x, so counts are cheaper as plain
    is_ge masks reduced on the idle TensorE via ones^T matmuls.
  - ACT runs ~0.92 ns/col + 185 ns per accumulator read.
  - GpSimd elementwise work and de-serialized engine overlap both
    inflate op latencies ~1.2-2.4x (SBUF port contention), so Pool
    stays idle and Ln runs IN PLACE after the masks (the dependency
    chain spreads SBUF pressure; a fully parallel variant measured
    slower).
  - The runtime preamble costs a fixed ~6.7 us, each dma_start ~0.6 us
    of serial SP dispatch; the io pool is deep enough that DMA issue
    never waits on compute.

Per sub-chunk (the identity ln(1)=0 masks without a select):
    a1 = (p-1)*t = y1-1  ->  sum ln(y1) = sum_{t==1} ln(p)   =: A1
    a0 = (t-1)*p = y0-1  ->  sum ln(y0) = sum_{t==0} ln(1-p) =: B0
  each one fused DVE scalar_tensor_tensor (int32 t converted on read).
  ACT computes Ln(a+1) in place with fused free-dim accumulation.
  Accuracy from exact f32 threshold counts, C1+C0-N:
    C1 = #(a1 >= -.5) = N0 + #(t1,p>=.5): DVE is_ge -> bf16 mask,
         partition-reduced by TensorE (ones^T @ mask) into PSUM.
    C0 = #(a0 >= -.5): ACT Sign(ln(y0)+ln2) accum on five middle
         2048-col sub-chunks (62% of columns), DVE is_ge masks on the
         ends AND the last 2048-col sub: ACT's per-sub work on sign
         subs (6.2 us) exceeds DVE's (5.5 us), so ACT builds lag that
         otherwise leaves it finishing ~4.5 us after DVE; keeping the
         tail sign-free lets both engines finish together (interleaved
         A/B measured -1.7 us).
  Sub-chunks are 1024 cols at both ends to shrink pipeline fill and
  drain; the work pool is 3 deep so DVE never stalls on ACT finishing
  a sub-chunk.  Activation biases are tracked tiles (no const-AP
  barrier) and the Ln table is pre-warmed behind the first DMA.
"""

import sys

if "/opt/trn_rl_repo" not in sys.path:
    sys.path.insert(0, "/opt/trn_rl_repo")

import numpy as np

import concourse.bass as bass
import concourse.bacc as bacc
import concourse.tile as tile
from concourse import mybir
from concourse.bass_utils import run_bass_kernel_spmd

N_CORES = 8
N = 16777216
P = 128
SHARD = N // N_CORES          # 2097152 elements per core
COLS = SHARD // P             # 16384 columns per core
SUBS = [1024, 1024, 2048, 2048, 2048, 2048, 2048, 2048, 1024, 1024]
assert sum(SUBS) == COLS
NS = len(SUBS)
# C0 via ACT Sign for the middle 2048-col subs, DVE is_ge on the ends
SIGN_SUBS = frozenset({2, 3, 4, 5, 6})
MMCOL = 512                   # matmul free-dim tile (one PSUM bank)
LN2 = 0.6931471805599453

AF = mybir.ActivationFunctionType
OP = mybir.AluOpType

# acc columns: [s] A1 sums, [NS+s] B0, [2NS+s] sign C0 (+/-1 sums,
# unused cols stay 0), [3NS] C1 fold, [3NS+1] DVE-C0 fold
ACC_COLS = 3 * NS + 2

_NC_CACHE = None


def build_bass():
    """Build the single-core Bass program (SPMD across 8 cores)."""
    global _NC_CACHE
    if _NC_CACHE is not None:
        return _NC_CACHE

    nc = bacc.Bacc("TRN2", target_bir_lowering=False, debug=False)

    p_in = nc.dram_tensor("p_in", [SHARD], mybir.dt.float32, kind="ExternalInput").ap()
    t_in = nc.dram_tensor("t_in", [SHARD], mybir.dt.int32, kind="ExternalInput").ap()
    acc_out = nc.dram_tensor("acc_out", [P, ACC_COLS], mybir.dt.float32, kind="ExternalOutput").ap()

    n_mm1 = COLS // MMCOL                                        # C1 matmuls
    dve_c0_cols = sum(SUBS[s] for s in range(NS) if s not in SIGN_SUBS)
    n_mm0 = dve_c0_cols // MMCOL                                 # C0 matmuls

    with tile.TileContext(nc) as tc:
        with (
            tc.tile_pool(name="io", bufs=8) as io_pool,
            tc.tile_pool(name="wk", bufs=3) as wk_pool,
            tc.tile_pool(name="jk", bufs=2) as jk_pool,
            tc.tile_pool(name="ps", bufs=1, space=bass.MemorySpace.PSUM) as psum_pool,
            tc.tile_pool(name="mi", bufs=1) as misc_pool,
        ):
            ones16 = misc_pool.tile([P, P], mybir.dt.bfloat16, tag="ones16")
            nc.gpsimd.memset(ones16[:], 1.0)
            co = misc_pool.tile([P, 1], mybir.dt.float32, tag="co")
            ln2c = misc_pool.tile([P, 1], mybir.dt.float32, tag="ln2c")
            nc.gpsimd.memset(co[:], 1.0)
            nc.gpsimd.memset(ln2c[:], LN2)
            warm = misc_pool.tile([P, 1], mybir.dt.float32, tag="warm")
            acc = misc_pool.tile([P, ACC_COLS], mybir.dt.float32, tag="acc")
            nc.gpsimd.memset(acc[:], 0.0)
            junk512 = misc_pool.tile([P, MMCOL], mybir.dt.float32, tag="junk512")
            ps1 = psum_pool.tile([P, MMCOL], mybir.dt.float32, tag="ps1")
            ps0 = psum_pool.tile([P, MMCOL], mybir.dt.float32, tag="ps0")

            # load the Ln/Sign table while the first DMA is in flight
            nc.scalar.activation(warm[:, 0:1], co[:, 0:1], AF.Ln, bias=co[:, 0:1])

            mm1 = mm0 = 0
            off = 0
            for s, C in enumerate(SUBS):
                p_t = io_pool.tile([P, C], mybir.dt.float32, tag="p")
                t_t = io_pool.tile([P, C], mybir.dt.int32, tag="t")
                nc.sync.dma_start(p_t[:], p_in[off : off + C * P].rearrange("(p f) -> p f", p=P))
                nc.sync.dma_start(t_t[:], t_in[off : off + C * P].rearrange("(p f) -> p f", p=P))
                off += C * P

                a1 = wk_pool.tile([P, C], mybir.dt.float32, tag="a1")
                a0 = wk_pool.tile([P, C], mybir.dt.float32, tag="a0")
                # a1 = (p-1)*t ; a0 = (t-1)*p
                nc.vector.scalar_tensor_tensor(a1[:], p_t[:], -1.0, t_t[:], OP.add, OP.mult)
                nc.vector.scalar_tensor_tensor(a0[:], t_t[:], -1.0, p_t[:], OP.add, OP.mult)

                # C1 mask (a1 >= -.5) at 2x into bf16, reduced on TensorE
                j1 = jk_pool.tile([P, C], mybir.dt.bfloat16, tag="j1")
                nc.vector.tensor_scalar(j1[:], a1[:], -0.5, None, OP.is_ge)
                for j in range(C // MMCOL):
                    nc.tensor.matmul(ps1[:], ones16[:], j1[:, j * MMCOL : (j + 1) * MMCOL],
                                     start=(mm1 == 0), stop=(mm1 == n_mm1 - 1))
                    mm1 += 1
                if s not in SIGN_SUBS:
                    jc = jk_pool.tile([P, C], mybir.dt.bfloat16, tag="jc")
                    nc.vector.tensor_scalar(jc[:], a0[:], -0.5, None, OP.is_ge)
                    for j in range(C // MMCOL):
                        nc.tensor.matmul(ps0[:], ones16[:], jc[:, j * MMCOL : (j + 1) * MMCOL],
                                         start=(mm0 == 0), stop=(mm0 == n_mm0 - 1))
                        mm0 += 1

                # in-place Ln with fused free-dim accumulation
                nc.scalar.activation(a1[:], a1[:], AF.Ln, bias=co[:, 0:1],
                                     accum_out=acc[:, s : s + 1])
                nc.scalar.activation(a0[:], a0[:], AF.Ln, bias=co[:, 0:1],
                                     accum_out=acc[:, NS + s : NS + s + 1])
                if s in SIGN_SUBS:
                    # post-Ln: sign(ln(y0)+ln2) = +/-1 for y0 >= .5 / < .5
                    js = jk_pool.tile([P, C], mybir.dt.bfloat16, tag="jc")
                    nc.scalar.activation(js[:], a0[:], AF.Sign, bias=ln2c[:, 0:1],
                                         accum_out=acc[:, 2 * NS + s : 2 * NS + s + 1])

            # fold the PSUM count matrices (128 identical rows) into columns
            nc.vector.tensor_scalar(junk512[:], ps1[:], 1.0 / P, None, OP.mult,
                                    OP.add, accum_out=acc[:, 3 * NS : 3 * NS + 1])
            nc.vector.tensor_scalar(junk512[:], ps0[:], 1.0 / P, None, OP.mult,
                                    OP.add, accum_out=acc[:, 3 * NS + 1 : 3 * NS + 2])

            nc.sync.dma_start(acc_out[:], acc[:])

    nc.finalize()
    _NC_CACHE = nc
    return nc


def make_in_maps(input, target):
    inp = np.ascontiguousarray(np.asarray(input, dtype=np.float32)).reshape(
        N_CORES, SHARD
    )
    tgt = np.ascontiguousarray(np.asarray(target, dtype=np.int32)).reshape(
        N_CORES, SHARD
    )
    return [{"p_in": inp[c], "t_in": tgt[c]} for c in range(N_CORES)]


def combine(results):
    """Host-side unshard: reduce the 8 cores' partial sums -> (loss, acc)."""
    A1 = B0 = S0 = C1 = C0m = 0.0
    sign_elems = sum(SUBS[s] for s in SIGN_SUBS) * P
    for r in results:
        a = np.asarray(r["acc_out"], dtype=np.float64)
        A1 += a[:, 0:NS].sum()
        B0 += a[:, NS : 2 * NS].sum()
        S0 += a[:, 2 * NS : 3 * NS].sum()
        C1 += a[:, 3 * NS].sum()
        C0m += a[:, 3 * NS + 1].sum()
    loss = -(1.6 * A1 + 0.4 * B0) / N
    C0 = (S0 + N_CORES * sign_elems) / 2.0 + C0m
    acc = (C1 + C0 - N) / N
    return np.float32(loss), np.float32(acc)


def run_on_hw(input, target, **spmd_kwargs):
    nc = build_bass()
    in_maps = make_in_maps(input, target)
    return run_bass_kernel_spmd(nc, in_maps, list(range(N_CORES)), **spmd_kwargs)


def kernel(input, target):
    br = run_on_hw(input, target)
    return combine(br.results)
